# revision 22
# baseline (speedup 1.0000x reference)
"""Trainium2 Bass kernel: 5x5 window median+variance denoise filter.

y = relu(x - noise_var/(var5x5(x)+1e-10) * (x - median5x5(x) + noise_bias))
with zero-padded 5x5 windows, unbiased variance (ddof=1).

Sharding: pure data parallel, B=16 images split 2-per-core across 8 cores.

v2: fp16 datapath. The median comparator network runs on DVE in fp16 to hit
the 2x_1p perf mode (all operands 2-byte, stride-1, 4B-aligned -- odd column
offsets are re-aligned via ACT-engine copies so no network op falls back to
1x). Squares, view-alignment copies, dtype conversions and the final relu
run on the otherwise-idle ACT engine. Variance accumulates in fp32 where it
matters (horizontal s25 sum, d/reciprocal path). Host pre-pads and converts
x to fp16; output returns fp16 and is upcast on host. Total numeric error
~1e-3 rel vs the 2e-2 harness gate.

Median via a pruned comparator network with shared column sorts:
  sort5 over the 5 dy-shifted planes (9 CE, shared by 5 horizontal windows)
  T = odd-even merge of adjacent sorted columns (13 CE, shared by 2 windows)
  final rank-12 selection from T(x-2), T(x), S(x+2) (35 CE, single-sided
  min/max pruned) -- 90 DVE min/max ops per full-image sweep, verified
  offline by exhaustive 0-1 principle.
"""
import numpy as np

import concourse.bass as bass  # noqa: F401
import concourse.mybir as mybir
from concourse import bacc, tile
from concourse.bass_utils import run_bass_kernel_spmd

F32 = mybir.dt.float32
F16 = mybir.dt.float16
ALU = mybir.AluOpType
ACTF = mybir.ActivationFunctionType

# (i, j, need_min, need_max) per structure; designed + 0/1-verified offline.
SORT5 = [(0, 1, 1, 1), (3, 4, 1, 1), (2, 4, 1, 1), (2, 3, 1, 1), (0, 3, 1, 1),
         (0, 2, 1, 1), (1, 4, 1, 1), (1, 3, 1, 1), (1, 2, 1, 1)]
T_CES = [(0, 5, 1, 1), (4, 9, 1, 1), (4, 5, 1, 1), (2, 7, 1, 1), (2, 4, 1, 1),
         (7, 5, 1, 1), (1, 6, 1, 1), (3, 8, 1, 1), (3, 6, 1, 1), (1, 2, 1, 1),
         (3, 4, 1, 1), (6, 7, 1, 1), (8, 5, 1, 1)]
F_CES = [(0, 10, 0, 1), (5, 15, 1, 0), (5, 10, 1, 1), (4, 14, 1, 1),
         (4, 5, 0, 1), (14, 10, 1, 0), (2, 12, 0, 1), (7, 17, 1, 0),
         (7, 12, 1, 1), (7, 5, 0, 1), (12, 14, 1, 1), (1, 11, 0, 1),
         (9, 19, 1, 0), (9, 11, 1, 1), (6, 16, 1, 1), (6, 9, 0, 1),
         (16, 11, 1, 0), (3, 13, 0, 1), (8, 18, 1, 0), (8, 13, 1, 1),
         (8, 9, 1, 1), (13, 16, 1, 0), (8, 5, 1, 1), (9, 12, 1, 1),
         (13, 14, 1, 1), (8, 20, 0, 1), (13, 24, 1, 0), (13, 20, 0, 1),
         (9, 22, 0, 1), (22, 20, 1, 0), (5, 21, 0, 1), (14, 21, 1, 0),
         (12, 23, 1, 0), (12, 14, 0, 1), (14, 22, 1, 0)]
F_OUT = 14


def schedule_ces(ces):
    """Reorder a CE list to avoid back-to-back producer->consumer ops on the
    in-order DVE (distance-1 RAW costs ~+150ns/op). Any permutation that
    preserves the relative order of CEs sharing a position has identical
    dataflow, so greedily pick ready CEs disjoint from the last emitted."""
    n = len(ces)
    preds = [set() for _ in range(n)]
    last_touch = {}
    for idx, ce in enumerate(ces):
        for p in ce[:2]:
            if p in last_touch:
                preds[idx].add(last_touch[p])
            last_touch[p] = idx
    emitted = [False] * n
    order = []
    hist = []          # positions of recently emitted CEs
    while len(order) < n:
        ready = [i for i in range(n) if not emitted[i]
                 and all(emitted[p] for p in preds[i])]
        pick = None
        for lookback in (2, 1, 0):
            recent = set().union(*hist[len(hist) - lookback:]) if lookback \
                else set()
            for i in ready:
                if not (set(ces[i][:2]) & recent):
                    pick = i
                    break
            if pick is not None:
                break
        emitted[pick] = True
        order.append(pick)
        hist.append(set(ces[pick][:2]))
    return [ces[i] for i in order]


SORT5 = schedule_ces(SORT5)
T_CES = schedule_ces(T_CES)
F_CES = schedule_ces(F_CES)

H = 512
W = 512
IMGS_PER_CORE = 2
N_CORES = 8
WIDE = W + 4          # 2-col halo each side
NBUF = 64             # cap on SBUF working buffers of [128, 2, WIDE] f16
NVB_COLS = 5          # nv, nb, c1=1/(24 nv), c2=1e-10/nv, sqrt(c1)/5


class BufPool:
    """Free-list over preallocated fixed SBUF tensors. Tile's dependency
    tracker makes reuse safe (WAR/RAW serialization on the same tensor)."""

    def __init__(self, nc):
        self.nc = nc
        self.bufs = []
        self.free = []

    def alloc(self):
        if self.free:
            return self.free.pop()
        idx = len(self.bufs)
        assert idx < NBUF, "SBUF buffer pool exhausted"
        t = self.nc.alloc_sbuf_tensor(f"wb{idx}", [128, 2, WIDE], F16).ap()
        self.bufs.append(t)
        return t

    def release(self, t):
        self.free.append(t)


class Wire:
    """SSA value living at column offset `off` of `buf`."""

    def __init__(self, buf, off, owned, pool, on_die=None):
        self.buf = buf
        self.off = off
        self.owned = owned      # release buf to pool when dead
        self.pool = pool
        self.reads_left = 0
        self.on_die = on_die

    def ap(self, width):
        return self.buf[:, :, self.off:self.off + width]

    def read_done(self):
        self.reads_left -= 1
        if self.reads_left == 0:
            self._die()

    def read_done_zero(self):
        if self.reads_left == 0:
            self._die()

    def _die(self):
        if self.owned:
            self.pool.release(self.buf)
        if self.on_die is not None:
            self.on_die()

    def detach_views(self, n_views):
        """Transfer buffer ownership to n_views future views; returns the
        on_die callback the views share. Call read_done() after to consume
        the terminal hold."""
        buf, owned, pool = self.buf, self.owned, self.pool
        self.owned = False
        state = {"n": n_views}

        def on_die():
            state["n"] -= 1
            if state["n"] == 0 and owned:
                pool.release(buf)
        return on_die


def run_stage(nc, pool, wires, ces, width, terminal_reads):
    """Emit one structure stage. A position's lifetime is split into segments
    at each rewrite; each Wire object gets the read count of its own segment
    so buffers release as soon as truly dead."""
    n = len(wires)
    # segs[i] = read counts per segment of position i (segment ends at a
    # write of i, which itself reads the old value).
    segs = [[] for _ in range(n)]
    cur = [0] * n
    for (a, b, nmin, nmax) in ces:
        cur[a] += 1
        cur[b] += 1
        if nmin:
            segs[a].append(cur[a])
            cur[a] = 0
        if nmax:
            segs[b].append(cur[b])
            cur[b] = 0
    for i in range(n):
        segs[i].append(cur[i] + terminal_reads.get(i, 0))

    seg_idx = [0] * n
    for i in range(n):
        wires[i].reads_left += segs[i][0]
        if segs[i][0] == 0:
            wires[i].read_done_zero()

    for (i, j, nmin, nmax) in ces:
        wi, wj = wires[i], wires[j]
        a = wi.ap(width)
        b = wj.ap(width)
        if nmin:
            lo = pool.alloc()
            nc.vector.tensor_tensor(lo[:, :, 0:width], a, b, ALU.min)
        if nmax:
            hi = pool.alloc()
            nc.vector.tensor_tensor(hi[:, :, 0:width], a, b, ALU.max)
        wi.read_done()
        wj.read_done()
        if nmin:
            seg_idx[i] += 1
            cnt = segs[i][seg_idx[i]]
            assert cnt > 0, "dead write (should be pruned offline)"
            wires[i] = Wire(lo, 0, True, pool)
            wires[i].reads_left = cnt
        if nmax:
            seg_idx[j] += 1
            cnt = segs[j][seg_idx[j]]
            assert cnt > 0, "dead write (should be pruned offline)"
            wires[j] = Wire(hi, 0, True, pool)
            wires[j].reads_left = cnt


def emit_chunk(nc, pool, f32bufs, tin, sq, out_tile, xa, ya, scal, img, half,
               probe=()):
    r0 = half * 256
    full = lambda t: t[:, :, :]
    nv_ap, nb_ap, c1_ap, c2_ap, sqc_ap = scal
    ident, P1, P2, V1sb, V2sb, q25c, s25c, tt = f32bufs

    # ---- loads: 5 dy-shifted fp16 tiles [128, 2, WIDE] from the pre-padded
    # shard (rows/cols already carry the 2-wide zero halo). Spread across the
    # three DGE queues (SP, ACT, GPSIMD) so transfers run in parallel. ----
    # NOTE: never issue DMA from the ACT queue -- its trigger would order
    # behind the whole per-chunk ACT program (which ends gated on DVE).
    if "noload" not in probe:
        load_eng = [nc.sync, nc.sync, nc.sync, nc.gpsimd, nc.gpsimd]
        for k, dy in enumerate(range(-2, 3)):
            for b in range(2):
                s = img * (H + 4) + r0 + b * 128 + dy + 2
                load_eng[k].dma_start(tin[k][:, b, :], xa[s: s + 128, :])

    # ---- x-plane vertical sums on PE (reads tin only; runs immediately),
    # then ACT: V1 downconvert, squares, V2 path. ----
    if "novar" not in probe:
        for b in range(2):
            for lo, hi in ((0, 512), (512, WIDE)):
                for k in range(5):
                    nc.tensor.matmul(P1[b][:, lo:hi], ident[:, :],
                                     tin[k][:, b, lo:hi],
                                     start=(k == 0), stop=(k == 4))
            nc.scalar.copy(V1sb[:, b, :], P1[b][:, :])
        for k in range(5):
            nc.scalar.square(full(sq[k]), full(tin[k]))
        for b in range(2):
            for lo, hi in ((0, 512), (512, WIDE)):
                for k in range(5):
                    nc.tensor.matmul(P2[b][:, lo:hi], ident[:, :],
                                     sq[k][:, b, lo:hi],
                                     start=(k == 0), stop=(k == 4))
            nc.scalar.copy(V2sb[:, b, :], P2[b][:, :])
        # horizontal 5-sums back into the same (now free) PSUM banks
        for (Vsb, Pp) in ((V1sb, P1), (V2sb, P2)):
            for b in range(2):
                for dx in range(5):
                    nc.tensor.matmul(Pp[b][:, 0:W], ident[:, :],
                                     Vsb[:, b, dx:dx + W],
                                     start=(dx == 0), stop=(dx == 4))
    # ---- median network part 1: sort5 (all fp16 2x; DVE starts here) ----
    if "nomed" not in probe:
        s_wires = [Wire(tin[k], 0, False, pool) for k in range(5)]
        run_stage(nc, pool, s_wires, SORT5, WIDE, {k: 2 for k in range(5)})

        # Sorted column planes r_k. Views: A_k = r_k @0 (width 515) and
        # C_k = r_k @4 (width 512) stay in place; B_k = r_k @1 is copied by
        # ACT into an aligned buffer so every T op keeps the 2x perf mode.
        t_wires = [None] * 10
        c_views = [None] * 5
        for k in range(5):
            rk = s_wires[k]
            bk = pool.alloc()
            nc.scalar.copy(bk[:, :, 0:515],
                           rk.buf[:, :, rk.off + 1:rk.off + 516])
            rk.read_done()      # the ACT copy consumed one terminal hold
            od = rk.detach_views(2)
            t_wires[k] = Wire(rk.buf, rk.off + 0, False, pool, on_die=od)
            c_views[k] = Wire(rk.buf, rk.off + 4, False, pool, on_die=od)
            t_wires[k + 5] = Wire(bk, 0, True, pool)
            rk.read_done()      # consume second terminal hold

    # q25c = c1*q25 + c2 ; s25c = (sqrt(c1)/5 * s25)^2 = c1*s25^2/25
    # (ACT, emitted after the B_k copies so those aren't delayed)
    if "novar" not in probe:
        for b in range(2):
            nc.scalar.activation(q25c[:, b, 0:W], P2[b][:, 0:W],
                                 ACTF.Identity, bias=c2_ap, scale=c1_ap)
            nc.scalar.activation(s25c[:, b, 0:W], P1[b][:, 0:W],
                                 ACTF.Square, scale=sqc_ap)

    # ---- median network part 2: T merge (width 516: tail lane is garbage
    # but never read downstream; keeps every op even-width/2x) ----
    if "nomed" not in probe:
        run_stage(nc, pool, t_wires, T_CES, WIDE, {j: 1 for j in range(10)})

    # ---- dd = q25c - s25c ; rcp = 1/dd  (2 DVE ops + ACT downconvert) ----
    if "novar" not in probe:
        nc.vector.tensor_tensor(tt[:, :, 0:W], q25c[:, :, 0:W],
                                s25c[:, :, 0:W], ALU.subtract)
        nc.vector.reciprocal_approx_fast(out=q25c[:, :, 0:W],
                                         in_=tt[:, :, 0:W])
        rcp = pool.alloc()   # fp16 copy of reciprocal for the 2x formula ops
        nc.scalar.copy(rcp[:, :, 0:W], q25c[:, :, 0:W])

    # ---- median network part 3: final selection ----
    if "nomed" not in probe:
        f_wires = [None] * 25
        for j in range(10):
            tw = t_wires[j]
            od = tw.detach_views(2)
            f_wires[j] = Wire(tw.buf, tw.off + 0, False, pool, on_die=od)
            f_wires[j + 10] = Wire(tw.buf, tw.off + 2, False, pool, on_die=od)
            tw.read_done()
        for k in range(5):
            f_wires[20 + k] = c_views[k]

        run_stage(nc, pool, f_wires, F_CES, W, {F_OUT: 1})
        mid = f_wires[F_OUT]

    # ---- formula: y = relu(x - rcp*((x + nb) - mid)), all fp16 2x ----
    xc = tin[2][:, :, 2:2 + W]              # center plane = x
    u = pool.alloc()
    if "nomed" in probe:
        mid_ap = tin[0][:, :, 2:2 + W]
    else:
        mid_ap = mid.ap(W)
    nc.vector.scalar_tensor_tensor(u[:, :, 0:W], xc, nb_ap, mid_ap,
                                   ALU.add, ALU.subtract)
    if "nomed" not in probe:
        mid.read_done()
    if "novar" not in probe:
        nc.vector.tensor_tensor(u[:, :, 0:W], rcp[:, :, 0:W], u[:, :, 0:W],
                                ALU.mult)
        pool.release(rcp)
    nc.vector.tensor_tensor(u[:, :, 0:W], xc, u[:, :, 0:W], ALU.subtract)
    nc.scalar.activation(out_tile[:, :, :], u[:, :, 0:W], ACTF.Relu)
    pool.release(u)

    # ---- store (fp16) ----
    for b in range(2):
        nc.gpsimd.dma_start(
            ya[img * H + r0 + b * 128: img * H + r0 + b * 128 + 128, :],
            out_tile[:, b, :],
        )


def build_module(repeat=1, hw_loop=None, probe=()):
    nc = bacc.Bacc(
        "TRN2",
        target_bir_lowering=False,
        debug=False,
        enable_asserts=False,
        num_devices=N_CORES,
    )
    x = nc.dram_tensor("x", [IMGS_PER_CORE, H + 4, WIDE], F16,
                       kind="ExternalInput")
    nvb = nc.dram_tensor("nvb", [128, NVB_COLS], F32, kind="ExternalInput")
    idm = nc.dram_tensor("ident", [128, 128], F16, kind="ExternalInput")
    y = nc.dram_tensor("y", [IMGS_PER_CORE, H, W], F16, kind="ExternalOutput")

    xa = x.ap().flatten_outer_dims()    # [2*516, 516] fp16
    ya = y.ap().flatten_outer_dims()

    with tile.TileContext(nc) as tc:
        pool = BufPool(nc)
        nvb_t = nc.alloc_sbuf_tensor("nvb_t", [128, NVB_COLS], F32).ap()
        nc.sync.dma_start(nvb_t[:, :], nvb.ap()[:, :])
        scal = tuple(nvb_t[:, i:i + 1] for i in range(NVB_COLS))
        ident = nc.alloc_sbuf_tensor("ident_t", [128, 128], F16).ap()
        nc.sync.dma_start(ident[:, :], idm.ap()[:, :])

        # double-buffered input/square/output tiles (chunk parity)
        tin = [[nc.alloc_sbuf_tensor(f"tin{p}_{k}", [128, 2, WIDE], F16).ap()
                for k in range(5)] for p in range(2)]
        sq = [[nc.alloc_sbuf_tensor(f"sq{p}_{k}", [128, 2, WIDE], F16).ap()
               for k in range(5)] for p in range(2)]
        out_t = [nc.alloc_sbuf_tensor(f"out{p}", [128, 2, W], F16).ap()
                 for p in range(2)]
        P1 = [nc.alloc_psum_tensor(f"P1b{b}", [128, WIDE], F32).ap()
              for b in range(2)]
        P2 = [nc.alloc_psum_tensor(f"P2b{b}", [128, WIDE], F32).ap()
              for b in range(2)]
        V1sb = nc.alloc_sbuf_tensor("V1sb", [128, 2, WIDE], F16).ap()
        V2sb = nc.alloc_sbuf_tensor("V2sb", [128, 2, WIDE], F16).ap()
        q25c = nc.alloc_sbuf_tensor("q25c", [128, 2, W], F32).ap()
        s25c = nc.alloc_sbuf_tensor("s25c", [128, 2, W], F32).ap()
        tt = nc.alloc_sbuf_tensor("ttb", [128, 2, W], F32).ap()
        f32bufs = (ident, P1, P2, V1sb, V2sb, q25c, s25c, tt)

        def body():
            for _ in range(repeat):
                for ci in range(2 * IMGS_PER_CORE):
                    img, half = divmod(ci, 2)
                    p = ci & 1
                    emit_chunk(nc, pool, f32bufs, tin[p], sq[p], out_t[p],
                               xa, ya, scal, img, half, probe=probe)

        if hw_loop is None:
            body()
        else:
            with tc.For_i(0, hw_loop, 1):
                body()

    nc.compile()
    return nc


_MODULE = None


def _get_module():
    global _MODULE
    if _MODULE is None:
        _MODULE = build_module()
    return _MODULE


def make_in_maps(x, nv, nb):
    """Host-side prep: pad + fp16-convert x, build per-core input maps."""
    nvb = np.empty((128, NVB_COLS), np.float32)
    c1 = 1.0 / (24.0 * nv)
    nvb[:, 0] = nv
    nvb[:, 1] = nb
    nvb[:, 2] = c1
    nvb[:, 3] = 1e-10 / nv
    nvb[:, 4] = np.sqrt(c1) / 5.0
    ident = np.eye(128, dtype=np.float16)

    B = x.shape[0]
    xpad = np.zeros((B, H + 4, WIDE), np.float16)
    xpad[:, 2:2 + H, 2:2 + W] = x[:, 0]
    in_maps = []
    for c in range(N_CORES):
        shard = np.ascontiguousarray(
            xpad[c * IMGS_PER_CORE:(c + 1) * IMGS_PER_CORE])
        in_maps.append({"x": shard, "nvb": nvb, "ident": ident})
    return in_maps


def kernel(x, noise_var, noise_bias):
    x = np.ascontiguousarray(np.asarray(x, dtype=np.float32))
    nv = float(np.asarray(noise_var).reshape(-1)[0])
    nb = float(np.asarray(noise_bias).reshape(-1)[0])
    B = x.shape[0]
    assert x.shape == (B, 1, H, W) and B == N_CORES * IMGS_PER_CORE

    nc = _get_module()
    in_maps = make_in_maps(x, nv, nb)
    res = run_bass_kernel_spmd(nc, in_maps, core_ids=list(range(N_CORES)))
    y = np.empty((B, 1, H, W), np.float32)
    for c in range(N_CORES):
        y[c * IMGS_PER_CORE:(c + 1) * IMGS_PER_CORE, 0] = \
            res.results[c]["y"].astype(np.float32)
    return y


# revision 25
# speedup vs baseline: 1.1245x; 1.1245x over previous
"""Trainium2 Bass kernel: 5x5 window median+variance denoise filter.

y = relu(x - noise_var/(var5x5(x)+1e-10) * (x - median5x5(x) + noise_bias))
with zero-padded 5x5 windows, unbiased variance (ddof=1).

Sharding: pure data parallel, B=16 images split 2-per-core across 8 cores.

v2: fp16 datapath. The median comparator network runs on DVE in fp16 to hit
the 2x_1p perf mode (all operands 2-byte, stride-1, 4B-aligned -- odd column
offsets are re-aligned via ACT-engine copies so no network op falls back to
1x). Squares, view-alignment copies, dtype conversions and the final relu
run on the otherwise-idle ACT engine. Variance accumulates in fp32 where it
matters (horizontal s25 sum, d/reciprocal path). Host pre-pads and converts
x to fp16; output returns fp16 and is upcast on host. Total numeric error
~1e-3 rel vs the 2e-2 harness gate.

Median via a pruned comparator network with shared column sorts:
  sort5 over the 5 dy-shifted planes (9 CE, shared by 5 horizontal windows)
  T = odd-even merge of adjacent sorted columns (13 CE, shared by 2 windows)
  final rank-12 selection from T(x-2), T(x), S(x+2) (35 CE, single-sided
  min/max pruned) -- 90 DVE min/max ops per full-image sweep, verified
  offline by exhaustive 0-1 principle.
"""
import numpy as np

import concourse.bass as bass  # noqa: F401
import concourse.mybir as mybir
from concourse import bacc, tile
from concourse.bass_utils import run_bass_kernel_spmd

F32 = mybir.dt.float32
F16 = mybir.dt.float16
ALU = mybir.AluOpType
ACTF = mybir.ActivationFunctionType

# (i, j, need_min, need_max) per structure; designed + 0/1-verified offline.
SORT5 = [(0, 1, 1, 1), (3, 4, 1, 1), (2, 4, 1, 1), (2, 3, 1, 1), (0, 3, 1, 1),
         (0, 2, 1, 1), (1, 4, 1, 1), (1, 3, 1, 1), (1, 2, 1, 1)]
T_CES = [(0, 5, 1, 1), (4, 9, 1, 1), (4, 5, 1, 1), (2, 7, 1, 1), (2, 4, 1, 1),
         (7, 5, 1, 1), (1, 6, 1, 1), (3, 8, 1, 1), (3, 6, 1, 1), (1, 2, 1, 1),
         (3, 4, 1, 1), (6, 7, 1, 1), (8, 5, 1, 1)]
F_CES = [(0, 10, 0, 1), (5, 15, 1, 0), (5, 10, 1, 1), (4, 14, 1, 1),
         (4, 5, 0, 1), (14, 10, 1, 0), (2, 12, 0, 1), (7, 17, 1, 0),
         (7, 12, 1, 1), (7, 5, 0, 1), (12, 14, 1, 1), (1, 11, 0, 1),
         (9, 19, 1, 0), (9, 11, 1, 1), (6, 16, 1, 1), (6, 9, 0, 1),
         (16, 11, 1, 0), (3, 13, 0, 1), (8, 18, 1, 0), (8, 13, 1, 1),
         (8, 9, 1, 1), (13, 16, 1, 0), (8, 5, 1, 1), (9, 12, 1, 1),
         (13, 14, 1, 1), (8, 20, 0, 1), (13, 24, 1, 0), (13, 20, 0, 1),
         (9, 22, 0, 1), (22, 20, 1, 0), (5, 21, 0, 1), (14, 21, 1, 0),
         (12, 23, 1, 0), (12, 14, 0, 1), (14, 22, 1, 0)]
F_OUT = 14


def schedule_ces(ces):
    """Reorder a CE list to avoid back-to-back producer->consumer ops on the
    in-order DVE (distance-1 RAW costs ~+150ns/op). Any permutation that
    preserves the relative order of CEs sharing a position has identical
    dataflow, so greedily pick ready CEs disjoint from the last emitted."""
    n = len(ces)
    preds = [set() for _ in range(n)]
    last_touch = {}
    for idx, ce in enumerate(ces):
        for p in ce[:2]:
            if p in last_touch:
                preds[idx].add(last_touch[p])
            last_touch[p] = idx
    emitted = [False] * n
    order = []
    hist = []          # positions of recently emitted CEs
    while len(order) < n:
        ready = [i for i in range(n) if not emitted[i]
                 and all(emitted[p] for p in preds[i])]
        pick = None
        for lookback in (2, 1, 0):
            recent = set().union(*hist[len(hist) - lookback:]) if lookback \
                else set()
            for i in ready:
                if not (set(ces[i][:2]) & recent):
                    pick = i
                    break
            if pick is not None:
                break
        emitted[pick] = True
        order.append(pick)
        hist.append(set(ces[pick][:2]))
    return [ces[i] for i in order]


SORT5 = schedule_ces(SORT5)
T_CES = schedule_ces(T_CES)
F_CES = schedule_ces(F_CES)

H = 512
W = 512
IMGS_PER_CORE = 2
N_CORES = 8
WIDE = W + 4          # 2-col halo each side
POOLW = WIDE + 2      # pool buffers padded so shifted views stay in-bounds
NBUF = 64             # cap on SBUF working buffers of [128, 2, POOLW] f16
NVB_COLS = 5          # nv, nb, c1=1/(24 nv), c2=1e-10/nv, sqrt(c1)/5


class BufPool:
    """Free-list over preallocated fixed SBUF tensors. Tile's dependency
    tracker makes reuse safe (WAR/RAW serialization on the same tensor)."""

    def __init__(self, nc):
        self.nc = nc
        self.bufs = []
        self.free = []

    def alloc(self):
        if self.free:
            return self.free.pop()
        idx = len(self.bufs)
        assert idx < NBUF, "SBUF buffer pool exhausted"
        t = self.nc.alloc_sbuf_tensor(f"wb{idx}", [128, 2, POOLW], F16).ap()
        self.bufs.append(t)
        return t

    def release(self, t):
        self.free.append(t)


class Wire:
    """SSA value living at column offset `off` of `buf`."""

    def __init__(self, buf, off, owned, pool, on_die=None):
        self.buf = buf
        self.off = off
        self.owned = owned      # release buf to pool when dead
        self.pool = pool
        self.reads_left = 0
        self.on_die = on_die

    def ap(self, width):
        return self.buf[:, :, self.off:self.off + width]

    def read_done(self):
        self.reads_left -= 1
        if self.reads_left == 0:
            self._die()

    def read_done_zero(self):
        if self.reads_left == 0:
            self._die()

    def _die(self):
        if self.owned:
            self.pool.release(self.buf)
        if self.on_die is not None:
            self.on_die()

    def detach_views(self, n_views):
        """Transfer buffer ownership to n_views future views; returns the
        on_die callback the views share. Call read_done() after to consume
        the terminal hold."""
        buf, owned, pool = self.buf, self.owned, self.pool
        self.owned = False
        state = {"n": n_views}

        def on_die():
            state["n"] -= 1
            if state["n"] == 0 and owned:
                pool.release(buf)
        return on_die


def run_stage(nc, pool, wires, ces, width, terminal_reads):
    """Emit one structure stage. A position's lifetime is split into segments
    at each rewrite; each Wire object gets the read count of its own segment
    so buffers release as soon as truly dead."""
    n = len(wires)
    # segs[i] = read counts per segment of position i (segment ends at a
    # write of i, which itself reads the old value).
    segs = [[] for _ in range(n)]
    cur = [0] * n
    for (a, b, nmin, nmax) in ces:
        cur[a] += 1
        cur[b] += 1
        if nmin:
            segs[a].append(cur[a])
            cur[a] = 0
        if nmax:
            segs[b].append(cur[b])
            cur[b] = 0
    for i in range(n):
        segs[i].append(cur[i] + terminal_reads.get(i, 0))

    seg_idx = [0] * n
    for i in range(n):
        wires[i].reads_left += segs[i][0]
        if segs[i][0] == 0:
            wires[i].read_done_zero()

    for (i, j, nmin, nmax) in ces:
        wi, wj = wires[i], wires[j]
        a = wi.ap(width)
        b = wj.ap(width)
        if nmin:
            lo = pool.alloc()
            nc.vector.tensor_tensor(lo[:, :, 0:width], a, b, ALU.min)
        if nmax:
            hi = pool.alloc()
            nc.vector.tensor_tensor(hi[:, :, 0:width], a, b, ALU.max)
        wi.read_done()
        wj.read_done()
        if nmin:
            seg_idx[i] += 1
            cnt = segs[i][seg_idx[i]]
            assert cnt > 0, "dead write (should be pruned offline)"
            wires[i] = Wire(lo, 0, True, pool)
            wires[i].reads_left = cnt
        if nmax:
            seg_idx[j] += 1
            cnt = segs[j][seg_idx[j]]
            assert cnt > 0, "dead write (should be pruned offline)"
            wires[j] = Wire(hi, 0, True, pool)
            wires[j].reads_left = cnt


def emit_chunk(nc, pool, f32bufs, tin, sq, out_tile, xa, ya, scal, img, half,
               probe=()):
    r0 = half * 256
    full = lambda t: t[:, :, :]
    nv_ap, nb_ap, c1_ap, c2_ap, sqc_ap = scal
    ident, P1, P2, V1sb, V2sb, q25c, s25c, tt = f32bufs

    # ---- loads: 5 dy-shifted fp16 tiles [128, 2, WIDE] from the pre-padded
    # shard (rows/cols already carry the 2-wide zero halo). Spread across the
    # three DGE queues (SP, ACT, GPSIMD) so transfers run in parallel. ----
    # NOTE: never issue DMA from the ACT queue -- its trigger would order
    # behind the whole per-chunk ACT program (which ends gated on DVE).
    if "noload" not in probe:
        load_eng = [nc.sync, nc.sync, nc.sync, nc.gpsimd, nc.gpsimd]
        for k, dy in enumerate(range(-2, 3)):
            for b in range(2):
                s = img * (H + 4) + r0 + b * 128 + dy + 2
                load_eng[k].dma_start(tin[k][:, b, :], xa[s: s + 128, :])

    # ---- x-plane vertical sums on PE (reads tin only; runs immediately),
    # then ACT: V1 downconvert, squares, V2 path. ----
    if "novar" not in probe:
        for b in range(2):
            for lo, hi in ((0, 512), (512, WIDE)):
                for k in range(5):
                    nc.tensor.matmul(P1[b][:, lo:hi], ident[:, :],
                                     tin[k][:, b, lo:hi],
                                     start=(k == 0), stop=(k == 4))
            nc.scalar.copy(V1sb[:, b, :], P1[b][:, :])
        for k in range(5):
            nc.scalar.square(full(sq[k]), full(tin[k]))
        for b in range(2):
            for lo, hi in ((0, 512), (512, WIDE)):
                for k in range(5):
                    nc.tensor.matmul(P2[b][:, lo:hi], ident[:, :],
                                     sq[k][:, b, lo:hi],
                                     start=(k == 0), stop=(k == 4))
            nc.scalar.copy(V2sb[:, b, :], P2[b][:, :])
        # horizontal 5-sums back into the same (now free) PSUM banks
        for (Vsb, Pp) in ((V1sb, P1), (V2sb, P2)):
            for b in range(2):
                for dx in range(5):
                    nc.tensor.matmul(Pp[b][:, 0:W], ident[:, :],
                                     Vsb[:, b, dx:dx + W],
                                     start=(dx == 0), stop=(dx == 4))
    # ---- median network part 1: sort5 (all fp16 2x; DVE starts here) ----
    if "nomed" not in probe:
        s_wires = [Wire(tin[k], 0, False, pool) for k in range(5)]
        run_stage(nc, pool, s_wires, SORT5, WIDE, {k: 1 for k in range(5)})

        # Sorted column planes r_k feed three shifted views each: A_k = @0,
        # B_k = @1 (odd offset costs only ~+60ns/op -- cheaper than copies),
        # C_k = @4.
        t_wires = [None] * 10
        c_views = [None] * 5
        for k in range(5):
            rk = s_wires[k]
            od = rk.detach_views(3)
            t_wires[k] = Wire(rk.buf, rk.off + 0, False, pool, on_die=od)
            t_wires[k + 5] = Wire(rk.buf, rk.off + 1, False, pool, on_die=od)
            c_views[k] = Wire(rk.buf, rk.off + 4, False, pool, on_die=od)
            rk.read_done()      # consume terminal hold

    # q25c = c1*q25 + c2 ; s25c = (sqrt(c1)/5 * s25)^2 = c1*s25^2/25
    # (ACT, emitted after the B_k copies so those aren't delayed)
    if "novar" not in probe:
        for b in range(2):
            nc.scalar.activation(q25c[:, b, 0:W], P2[b][:, 0:W],
                                 ACTF.Identity, bias=c2_ap, scale=c1_ap)
            nc.scalar.activation(s25c[:, b, 0:W], P1[b][:, 0:W],
                                 ACTF.Square, scale=sqc_ap)

    # ---- median network part 2: T merge (width 516: tail lane is garbage
    # but never read downstream; keeps every op even-width/2x) ----
    if "nomed" not in probe:
        run_stage(nc, pool, t_wires, T_CES, WIDE, {j: 1 for j in range(10)})

    # ---- dd = q25c - s25c ; rcp = 1/dd  (2 DVE ops + ACT downconvert) ----
    if "novar" not in probe:
        nc.vector.tensor_tensor(tt[:, :, 0:W], q25c[:, :, 0:W],
                                s25c[:, :, 0:W], ALU.subtract)
        nc.vector.reciprocal_approx_fast(out=q25c[:, :, 0:W],
                                         in_=tt[:, :, 0:W])
        rcp = pool.alloc()   # fp16 copy of reciprocal for the 2x formula ops
        nc.scalar.copy(rcp[:, :, 0:W], q25c[:, :, 0:W])

    # ---- median network part 3: final selection ----
    if "nomed" not in probe:
        f_wires = [None] * 25
        for j in range(10):
            tw = t_wires[j]
            od = tw.detach_views(2)
            f_wires[j] = Wire(tw.buf, tw.off + 0, False, pool, on_die=od)
            f_wires[j + 10] = Wire(tw.buf, tw.off + 2, False, pool, on_die=od)
            tw.read_done()
        for k in range(5):
            f_wires[20 + k] = c_views[k]

        run_stage(nc, pool, f_wires, F_CES, W, {F_OUT: 1})
        mid = f_wires[F_OUT]

    # ---- formula: y = relu(x - rcp*((x + nb) - mid)), all fp16 2x ----
    xc = tin[2][:, :, 2:2 + W]              # center plane = x
    u = pool.alloc()
    if "nomed" in probe:
        mid_ap = tin[0][:, :, 2:2 + W]
    else:
        mid_ap = mid.ap(W)
    nc.vector.scalar_tensor_tensor(u[:, :, 0:W], xc, nb_ap, mid_ap,
                                   ALU.add, ALU.subtract)
    if "nomed" not in probe:
        mid.read_done()
    if "novar" not in probe:
        nc.vector.tensor_tensor(u[:, :, 0:W], rcp[:, :, 0:W], u[:, :, 0:W],
                                ALU.mult)
        pool.release(rcp)
    nc.vector.tensor_tensor(u[:, :, 0:W], xc, u[:, :, 0:W], ALU.subtract)
    nc.scalar.activation(out_tile[:, :, :], u[:, :, 0:W], ACTF.Relu)
    pool.release(u)

    # ---- store (fp16) ----
    for b in range(2):
        nc.gpsimd.dma_start(
            ya[img * H + r0 + b * 128: img * H + r0 + b * 128 + 128, :],
            out_tile[:, b, :],
        )


def build_module(repeat=1, hw_loop=None, probe=()):
    nc = bacc.Bacc(
        "TRN2",
        target_bir_lowering=False,
        debug=False,
        enable_asserts=False,
        num_devices=N_CORES,
    )
    x = nc.dram_tensor("x", [IMGS_PER_CORE, H + 4, WIDE], F16,
                       kind="ExternalInput")
    nvb = nc.dram_tensor("nvb", [128, NVB_COLS], F32, kind="ExternalInput")
    idm = nc.dram_tensor("ident", [128, 128], F16, kind="ExternalInput")
    y = nc.dram_tensor("y", [IMGS_PER_CORE, H, W], F16, kind="ExternalOutput")

    xa = x.ap().flatten_outer_dims()    # [2*516, 516] fp16
    ya = y.ap().flatten_outer_dims()

    with tile.TileContext(nc) as tc:
        pool = BufPool(nc)
        nvb_t = nc.alloc_sbuf_tensor("nvb_t", [128, NVB_COLS], F32).ap()
        nc.sync.dma_start(nvb_t[:, :], nvb.ap()[:, :])
        scal = tuple(nvb_t[:, i:i + 1] for i in range(NVB_COLS))
        ident = nc.alloc_sbuf_tensor("ident_t", [128, 128], F16).ap()
        nc.sync.dma_start(ident[:, :], idm.ap()[:, :])

        # double-buffered input/square/output tiles (chunk parity)
        tin = [[nc.alloc_sbuf_tensor(f"tin{p}_{k}", [128, 2, WIDE], F16).ap()
                for k in range(5)] for p in range(2)]
        sq = [[nc.alloc_sbuf_tensor(f"sq{p}_{k}", [128, 2, WIDE], F16).ap()
               for k in range(5)] for p in range(2)]
        out_t = [nc.alloc_sbuf_tensor(f"out{p}", [128, 2, W], F16).ap()
                 for p in range(2)]
        P1 = [nc.alloc_psum_tensor(f"P1b{b}", [128, WIDE], F32).ap()
              for b in range(2)]
        P2 = [nc.alloc_psum_tensor(f"P2b{b}", [128, WIDE], F32).ap()
              for b in range(2)]
        V1sb = nc.alloc_sbuf_tensor("V1sb", [128, 2, WIDE], F16).ap()
        V2sb = nc.alloc_sbuf_tensor("V2sb", [128, 2, WIDE], F16).ap()
        q25c = nc.alloc_sbuf_tensor("q25c", [128, 2, W], F32).ap()
        s25c = nc.alloc_sbuf_tensor("s25c", [128, 2, W], F32).ap()
        tt = nc.alloc_sbuf_tensor("ttb", [128, 2, W], F32).ap()
        f32bufs = (ident, P1, P2, V1sb, V2sb, q25c, s25c, tt)

        def body():
            for _ in range(repeat):
                for ci in range(2 * IMGS_PER_CORE):
                    img, half = divmod(ci, 2)
                    p = ci & 1
                    emit_chunk(nc, pool, f32bufs, tin[p], sq[p], out_t[p],
                               xa, ya, scal, img, half, probe=probe)

        if hw_loop is None:
            body()
        else:
            with tc.For_i(0, hw_loop, 1):
                body()

    nc.compile()
    return nc


_MODULE = None


def _get_module():
    global _MODULE
    if _MODULE is None:
        _MODULE = build_module()
    return _MODULE


def make_in_maps(x, nv, nb):
    """Host-side prep: pad + fp16-convert x, build per-core input maps."""
    nvb = np.empty((128, NVB_COLS), np.float32)
    c1 = 1.0 / (24.0 * nv)
    nvb[:, 0] = nv
    nvb[:, 1] = nb
    nvb[:, 2] = c1
    nvb[:, 3] = 1e-10 / nv
    nvb[:, 4] = np.sqrt(c1) / 5.0
    ident = np.eye(128, dtype=np.float16)

    B = x.shape[0]
    xpad = np.zeros((B, H + 4, WIDE), np.float16)
    xpad[:, 2:2 + H, 2:2 + W] = x[:, 0]
    in_maps = []
    for c in range(N_CORES):
        shard = np.ascontiguousarray(
            xpad[c * IMGS_PER_CORE:(c + 1) * IMGS_PER_CORE])
        in_maps.append({"x": shard, "nvb": nvb, "ident": ident})
    return in_maps


def kernel(x, noise_var, noise_bias):
    x = np.ascontiguousarray(np.asarray(x, dtype=np.float32))
    nv = float(np.asarray(noise_var).reshape(-1)[0])
    nb = float(np.asarray(noise_bias).reshape(-1)[0])
    B = x.shape[0]
    assert x.shape == (B, 1, H, W) and B == N_CORES * IMGS_PER_CORE

    nc = _get_module()
    in_maps = make_in_maps(x, nv, nb)
    res = run_bass_kernel_spmd(nc, in_maps, core_ids=list(range(N_CORES)))
    y = np.empty((B, 1, H, W), np.float32)
    for c in range(N_CORES):
        y[c * IMGS_PER_CORE:(c + 1) * IMGS_PER_CORE, 0] = \
            res.results[c]["y"].astype(np.float32)
    return y


# revision 26
# speedup vs baseline: 1.3694x; 1.2178x over previous
"""Trainium2 Bass kernel: 5x5 window median+variance denoise filter.

y = relu(x - noise_var/(var5x5(x)+1e-10) * (x - median5x5(x) + noise_bias))
with zero-padded 5x5 windows, unbiased variance (ddof=1).

Sharding: pure data parallel, B=16 images split 2-per-core across 8 cores.

v2: fp16 datapath. The median comparator network runs on DVE in fp16 to hit
the 2x_1p perf mode (all operands 2-byte, stride-1, 4B-aligned -- odd column
offsets are re-aligned via ACT-engine copies so no network op falls back to
1x). Squares, view-alignment copies, dtype conversions and the final relu
run on the otherwise-idle ACT engine. Variance accumulates in fp32 where it
matters (horizontal s25 sum, d/reciprocal path). Host pre-pads and converts
x to fp16; output returns fp16 and is upcast on host. Total numeric error
~1e-3 rel vs the 2e-2 harness gate.

Median via a pruned comparator network with shared column sorts:
  sort5 over the 5 dy-shifted planes (9 CE, shared by 5 horizontal windows)
  T = odd-even merge of adjacent sorted columns (13 CE, shared by 2 windows)
  final rank-12 selection from T(x-2), T(x), S(x+2) (35 CE, single-sided
  min/max pruned) -- 90 DVE min/max ops per full-image sweep, verified
  offline by exhaustive 0-1 principle.
"""
import numpy as np

import concourse.bass as bass  # noqa: F401
import concourse.mybir as mybir
from concourse import bacc, tile
from concourse.bass_utils import run_bass_kernel_spmd

F32 = mybir.dt.float32
F16 = mybir.dt.float16
ALU = mybir.AluOpType
ACTF = mybir.ActivationFunctionType

# (i, j, need_min, need_max) per structure; designed + 0/1-verified offline.
SORT5 = [(0, 1, 1, 1), (3, 4, 1, 1), (2, 4, 1, 1), (2, 3, 1, 1), (0, 3, 1, 1),
         (0, 2, 1, 1), (1, 4, 1, 1), (1, 3, 1, 1), (1, 2, 1, 1)]
T_CES = [(0, 5, 1, 1), (4, 9, 1, 1), (4, 5, 1, 1), (2, 7, 1, 1), (2, 4, 1, 1),
         (7, 5, 1, 1), (1, 6, 1, 1), (3, 8, 1, 1), (3, 6, 1, 1), (1, 2, 1, 1),
         (3, 4, 1, 1), (6, 7, 1, 1), (8, 5, 1, 1)]
F_CES = [(0, 10, 0, 1), (5, 15, 1, 0), (5, 10, 1, 1), (4, 14, 1, 1),
         (4, 5, 0, 1), (14, 10, 1, 0), (2, 12, 0, 1), (7, 17, 1, 0),
         (7, 12, 1, 1), (7, 5, 0, 1), (12, 14, 1, 1), (1, 11, 0, 1),
         (9, 19, 1, 0), (9, 11, 1, 1), (6, 16, 1, 1), (6, 9, 0, 1),
         (16, 11, 1, 0), (3, 13, 0, 1), (8, 18, 1, 0), (8, 13, 1, 1),
         (8, 9, 1, 1), (13, 16, 1, 0), (8, 5, 1, 1), (9, 12, 1, 1),
         (13, 14, 1, 1), (8, 20, 0, 1), (13, 24, 1, 0), (13, 20, 0, 1),
         (9, 22, 0, 1), (22, 20, 1, 0), (5, 21, 0, 1), (14, 21, 1, 0),
         (12, 23, 1, 0), (12, 14, 0, 1), (14, 22, 1, 0)]
F_OUT = 14


def schedule_ces(ces):
    """Reorder a CE list to avoid back-to-back producer->consumer ops on the
    in-order DVE (distance-1 RAW costs ~+150ns/op). Any permutation that
    preserves the relative order of CEs sharing a position has identical
    dataflow, so greedily pick ready CEs disjoint from the last emitted."""
    n = len(ces)
    preds = [set() for _ in range(n)]
    last_touch = {}
    for idx, ce in enumerate(ces):
        for p in ce[:2]:
            if p in last_touch:
                preds[idx].add(last_touch[p])
            last_touch[p] = idx
    emitted = [False] * n
    order = []
    hist = []          # positions of recently emitted CEs
    while len(order) < n:
        ready = [i for i in range(n) if not emitted[i]
                 and all(emitted[p] for p in preds[i])]
        pick = None
        for lookback in (2, 1, 0):
            recent = set().union(*hist[len(hist) - lookback:]) if lookback \
                else set()
            for i in ready:
                if not (set(ces[i][:2]) & recent):
                    pick = i
                    break
            if pick is not None:
                break
        emitted[pick] = True
        order.append(pick)
        hist.append(set(ces[pick][:2]))
    return [ces[i] for i in order]


SORT5 = schedule_ces(SORT5)
T_CES = schedule_ces(T_CES)
F_CES = schedule_ces(F_CES)

H = 512
W = 512
IMGS_PER_CORE = 2
N_CORES = 8
WIDE = W + 4          # 2-col halo each side
POOLW = WIDE + 2      # pool buffers padded so shifted views stay in-bounds
NBUF = 64             # cap on SBUF working buffers of [128, 2, POOLW] f16
NVB_COLS = 5          # nv, nb, c1=1/(24 nv), c2=1e-10/nv, sqrt(c1)/5


class BufPool:
    """Free-list over preallocated fixed SBUF tensors. Tile's dependency
    tracker makes reuse safe (WAR/RAW serialization on the same tensor)."""

    def __init__(self, nc):
        self.nc = nc
        self.bufs = []
        self.free = []

    def alloc(self):
        if self.free:
            return self.free.pop()
        idx = len(self.bufs)
        assert idx < NBUF, "SBUF buffer pool exhausted"
        t = self.nc.alloc_sbuf_tensor(f"wb{idx}", [128, 2, POOLW], F16).ap()
        self.bufs.append(t)
        return t

    def release(self, t):
        self.free.append(t)


class Wire:
    """SSA value living at column offset `off` of `buf`."""

    def __init__(self, buf, off, owned, pool, on_die=None):
        self.buf = buf
        self.off = off
        self.owned = owned      # release buf to pool when dead
        self.pool = pool
        self.reads_left = 0
        self.on_die = on_die

    def ap(self, width):
        return self.buf[:, :, self.off:self.off + width]

    def read_done(self):
        self.reads_left -= 1
        if self.reads_left == 0:
            self._die()

    def read_done_zero(self):
        if self.reads_left == 0:
            self._die()

    def _die(self):
        if self.owned:
            self.pool.release(self.buf)
        if self.on_die is not None:
            self.on_die()

    def detach_views(self, n_views):
        """Transfer buffer ownership to n_views future views; returns the
        on_die callback the views share. Call read_done() after to consume
        the terminal hold."""
        buf, owned, pool = self.buf, self.owned, self.pool
        self.owned = False
        state = {"n": n_views}

        def on_die():
            state["n"] -= 1
            if state["n"] == 0 and owned:
                pool.release(buf)
        return on_die


def run_stage(nc, pool, wires, ces, width, terminal_reads):
    """Emit one structure stage. A position's lifetime is split into segments
    at each rewrite; each Wire object gets the read count of its own segment
    so buffers release as soon as truly dead."""
    n = len(wires)
    # segs[i] = read counts per segment of position i (segment ends at a
    # write of i, which itself reads the old value).
    segs = [[] for _ in range(n)]
    cur = [0] * n
    for (a, b, nmin, nmax) in ces:
        cur[a] += 1
        cur[b] += 1
        if nmin:
            segs[a].append(cur[a])
            cur[a] = 0
        if nmax:
            segs[b].append(cur[b])
            cur[b] = 0
    for i in range(n):
        segs[i].append(cur[i] + terminal_reads.get(i, 0))

    seg_idx = [0] * n
    for i in range(n):
        wires[i].reads_left += segs[i][0]
        if segs[i][0] == 0:
            wires[i].read_done_zero()

    for (i, j, nmin, nmax) in ces:
        wi, wj = wires[i], wires[j]
        a = wi.ap(width)
        b = wj.ap(width)
        if nmin:
            lo = pool.alloc()
            nc.vector.tensor_tensor(lo[:, :, 0:width], a, b, ALU.min)
        if nmax:
            hi = pool.alloc()
            nc.vector.tensor_tensor(hi[:, :, 0:width], a, b, ALU.max)
        wi.read_done()
        wj.read_done()
        if nmin:
            seg_idx[i] += 1
            cnt = segs[i][seg_idx[i]]
            assert cnt > 0, "dead write (should be pruned offline)"
            wires[i] = Wire(lo, 0, True, pool)
            wires[i].reads_left = cnt
        if nmax:
            seg_idx[j] += 1
            cnt = segs[j][seg_idx[j]]
            assert cnt > 0, "dead write (should be pruned offline)"
            wires[j] = Wire(hi, 0, True, pool)
            wires[j].reads_left = cnt


def emit_chunk(nc, pool, f32bufs, tin, sq, out_tile, xa, ya, scal, img, half,
               probe=()):
    r0 = half * 256
    full = lambda t: t[:, :, :]
    nv_ap, nb_ap, c1_ap, c2_ap, sqc_ap = scal
    ident, P1, P2, V1sb, V2sb, q25c, s25c, tt = f32bufs

    # ---- loads: 5 dy-shifted fp16 tiles [128, 2, WIDE] from the pre-padded
    # shard (rows/cols already carry the 2-wide zero halo). Spread across the
    # three DGE queues (SP, ACT, GPSIMD) so transfers run in parallel. ----
    # NOTE: never issue DMA from the ACT queue -- its trigger would order
    # behind the whole per-chunk ACT program (which ends gated on DVE).
    if "noload" not in probe:
        for k, dy in enumerate(range(-2, 3)):
            for b in range(2):
                s = img * (H + 4) + r0 + b * 128 + dy + 2
                eng = nc.sync if (k + b) % 2 == 0 else nc.gpsimd
                eng.dma_start(tin[k][:, b, :], xa[s: s + 128, :])

    # ---- x-plane vertical sums on PE (reads tin only; runs immediately),
    # then ACT: V1 downconvert, squares, V2 path. ----
    if "novar" not in probe:
        for b in range(2):
            for lo, hi in ((0, 512), (512, WIDE)):
                for k in range(5):
                    nc.tensor.matmul(P1[b][:, lo:hi], ident[:, :],
                                     tin[k][:, b, lo:hi],
                                     start=(k == 0), stop=(k == 4))
            nc.scalar.copy(V1sb[:, b, :], P1[b][:, :])
        for k in range(5):
            nc.scalar.square(full(sq[k]), full(tin[k]))
        for b in range(2):
            for lo, hi in ((0, 512), (512, WIDE)):
                for k in range(5):
                    nc.tensor.matmul(P2[b][:, lo:hi], ident[:, :],
                                     sq[k][:, b, lo:hi],
                                     start=(k == 0), stop=(k == 4))
            nc.scalar.copy(V2sb[:, b, :], P2[b][:, :])
        # horizontal 5-sums back into the same (now free) PSUM banks
        for (Vsb, Pp) in ((V1sb, P1), (V2sb, P2)):
            for b in range(2):
                for dx in range(5):
                    nc.tensor.matmul(Pp[b][:, 0:W], ident[:, :],
                                     Vsb[:, b, dx:dx + W],
                                     start=(dx == 0), stop=(dx == 4))
    # ---- median network part 1: sort5 (all fp16 2x; DVE starts here) ----
    if "nomed" not in probe:
        s_wires = [Wire(tin[k], 0, False, pool) for k in range(5)]
        run_stage(nc, pool, s_wires, SORT5, WIDE, {k: 1 for k in range(5)})

        # Sorted column planes r_k feed three shifted views each: A_k = @0,
        # B_k = @1 (odd offset costs only ~+60ns/op -- cheaper than copies),
        # C_k = @4.
        t_wires = [None] * 10
        c_views = [None] * 5
        for k in range(5):
            rk = s_wires[k]
            od = rk.detach_views(3)
            t_wires[k] = Wire(rk.buf, rk.off + 0, False, pool, on_die=od)
            t_wires[k + 5] = Wire(rk.buf, rk.off + 1, False, pool, on_die=od)
            c_views[k] = Wire(rk.buf, rk.off + 4, False, pool, on_die=od)
            rk.read_done()      # consume terminal hold

    # q25c = c1*q25 + c2 ; s25c = (sqrt(c1)/5 * s25)^2 = c1*s25^2/25
    # (ACT, emitted after the B_k copies so those aren't delayed)
    if "novar" not in probe:
        for b in range(2):
            nc.scalar.activation(q25c[:, b, 0:W], P2[b][:, 0:W],
                                 ACTF.Identity, bias=c2_ap, scale=c1_ap)
            nc.scalar.activation(s25c[:, b, 0:W], P1[b][:, 0:W],
                                 ACTF.Square, scale=sqc_ap)

    # ---- median network part 2: T merge (width 516: tail lane is garbage
    # but never read downstream; keeps every op even-width/2x) ----
    if "nomed" not in probe:
        run_stage(nc, pool, t_wires, T_CES, WIDE, {j: 1 for j in range(10)})

    # ---- dd = q25c - s25c ; rcp = 1/dd  (2 DVE ops + ACT downconvert) ----
    if "novar" not in probe:
        nc.vector.tensor_tensor(tt[:, :, 0:W], q25c[:, :, 0:W],
                                s25c[:, :, 0:W], ALU.subtract)
        nc.vector.reciprocal_approx_fast(out=q25c[:, :, 0:W],
                                         in_=tt[:, :, 0:W])
        rcp = pool.alloc()   # fp16 copy of reciprocal for the 2x formula ops
        nc.scalar.copy(rcp[:, :, 0:W], q25c[:, :, 0:W])

    # ---- median network part 3: final selection ----
    if "nomed" not in probe:
        f_wires = [None] * 25
        for j in range(10):
            tw = t_wires[j]
            od = tw.detach_views(2)
            f_wires[j] = Wire(tw.buf, tw.off + 0, False, pool, on_die=od)
            f_wires[j + 10] = Wire(tw.buf, tw.off + 2, False, pool, on_die=od)
            tw.read_done()
        for k in range(5):
            f_wires[20 + k] = c_views[k]

        run_stage(nc, pool, f_wires, F_CES, W, {F_OUT: 1})
        mid = f_wires[F_OUT]

    # ---- formula: y = relu(x - rcp*((x + nb) - mid)), all fp16 2x ----
    xc = tin[2][:, :, 2:2 + W]              # center plane = x
    u = pool.alloc()
    if "nomed" in probe:
        mid_ap = tin[0][:, :, 2:2 + W]
    else:
        mid_ap = mid.ap(W)
    nc.vector.scalar_tensor_tensor(u[:, :, 0:W], xc, nb_ap, mid_ap,
                                   ALU.add, ALU.subtract)
    if "nomed" not in probe:
        mid.read_done()
    if "novar" not in probe:
        nc.vector.tensor_tensor(u[:, :, 0:W], rcp[:, :, 0:W], u[:, :, 0:W],
                                ALU.mult)
        pool.release(rcp)
    nc.vector.tensor_tensor(u[:, :, 0:W], xc, u[:, :, 0:W], ALU.subtract)
    nc.scalar.activation(out_tile[:, :, :], u[:, :, 0:W], ACTF.Relu)
    pool.release(u)

    # ---- store (fp16) ----
    for b in range(2):
        nc.gpsimd.dma_start(
            ya[img * H + r0 + b * 128: img * H + r0 + b * 128 + 128, :],
            out_tile[:, b, :],
        )


def build_module(repeat=1, hw_loop=None, probe=()):
    nc = bacc.Bacc(
        "TRN2",
        target_bir_lowering=False,
        debug=False,
        enable_asserts=False,
        num_devices=N_CORES,
    )
    x = nc.dram_tensor("x", [IMGS_PER_CORE, H + 4, WIDE], F16,
                       kind="ExternalInput")
    nvb = nc.dram_tensor("nvb", [128, NVB_COLS], F32, kind="ExternalInput")
    idm = nc.dram_tensor("ident", [128, 128], F16, kind="ExternalInput")
    y = nc.dram_tensor("y", [IMGS_PER_CORE, H, W], F16, kind="ExternalOutput")

    xa = x.ap().flatten_outer_dims()    # [2*516, 516] fp16
    ya = y.ap().flatten_outer_dims()

    with tile.TileContext(nc) as tc:
        pool = BufPool(nc)
        nvb_t = nc.alloc_sbuf_tensor("nvb_t", [128, NVB_COLS], F32).ap()
        nc.sync.dma_start(nvb_t[:, :], nvb.ap()[:, :])
        scal = tuple(nvb_t[:, i:i + 1] for i in range(NVB_COLS))
        ident = nc.alloc_sbuf_tensor("ident_t", [128, 128], F16).ap()
        nc.sync.dma_start(ident[:, :], idm.ap()[:, :])

        # double-buffered input/square/output tiles (chunk parity)
        tin = [[nc.alloc_sbuf_tensor(f"tin{p}_{k}", [128, 2, WIDE], F16).ap()
                for k in range(5)] for p in range(2)]
        sq = [[nc.alloc_sbuf_tensor(f"sq{p}_{k}", [128, 2, WIDE], F16).ap()
               for k in range(5)] for p in range(2)]
        out_t = [nc.alloc_sbuf_tensor(f"out{p}", [128, 2, W], F16).ap()
                 for p in range(2)]
        P1 = [nc.alloc_psum_tensor(f"P1b{b}", [128, WIDE], F32).ap()
              for b in range(2)]
        P2 = [nc.alloc_psum_tensor(f"P2b{b}", [128, WIDE], F32).ap()
              for b in range(2)]
        V1sb = nc.alloc_sbuf_tensor("V1sb", [128, 2, WIDE], F16).ap()
        V2sb = nc.alloc_sbuf_tensor("V2sb", [128, 2, WIDE], F16).ap()
        q25c = nc.alloc_sbuf_tensor("q25c", [128, 2, W], F32).ap()
        s25c = nc.alloc_sbuf_tensor("s25c", [128, 2, W], F32).ap()
        tt = nc.alloc_sbuf_tensor("ttb", [128, 2, W], F32).ap()
        f32bufs = (ident, P1, P2, V1sb, V2sb, q25c, s25c, tt)

        def body():
            for _ in range(repeat):
                for ci in range(2 * IMGS_PER_CORE):
                    img, half = divmod(ci, 2)
                    p = ci & 1
                    emit_chunk(nc, pool, f32bufs, tin[p], sq[p], out_t[p],
                               xa, ya, scal, img, half, probe=probe)

        if hw_loop is None:
            body()
        else:
            with tc.For_i(0, hw_loop, 1):
                body()

    nc.compile()
    return nc


_MODULE = None


def _get_module():
    global _MODULE
    if _MODULE is None:
        _MODULE = build_module()
    return _MODULE


def make_in_maps(x, nv, nb):
    """Host-side prep: pad + fp16-convert x, build per-core input maps."""
    nvb = np.empty((128, NVB_COLS), np.float32)
    c1 = 1.0 / (24.0 * nv)
    nvb[:, 0] = nv
    nvb[:, 1] = nb
    nvb[:, 2] = c1
    nvb[:, 3] = 1e-10 / nv
    nvb[:, 4] = np.sqrt(c1) / 5.0
    ident = np.eye(128, dtype=np.float16)

    B = x.shape[0]
    xpad = np.zeros((B, H + 4, WIDE), np.float16)
    xpad[:, 2:2 + H, 2:2 + W] = x[:, 0]
    in_maps = []
    for c in range(N_CORES):
        shard = np.ascontiguousarray(
            xpad[c * IMGS_PER_CORE:(c + 1) * IMGS_PER_CORE])
        in_maps.append({"x": shard, "nvb": nvb, "ident": ident})
    return in_maps


def kernel(x, noise_var, noise_bias):
    x = np.ascontiguousarray(np.asarray(x, dtype=np.float32))
    nv = float(np.asarray(noise_var).reshape(-1)[0])
    nb = float(np.asarray(noise_bias).reshape(-1)[0])
    B = x.shape[0]
    assert x.shape == (B, 1, H, W) and B == N_CORES * IMGS_PER_CORE

    nc = _get_module()
    in_maps = make_in_maps(x, nv, nb)
    res = run_bass_kernel_spmd(nc, in_maps, core_ids=list(range(N_CORES)))
    y = np.empty((B, 1, H, W), np.float32)
    for c in range(N_CORES):
        y[c * IMGS_PER_CORE:(c + 1) * IMGS_PER_CORE, 0] = \
            res.results[c]["y"].astype(np.float32)
    return y


# revision 28
# speedup vs baseline: 1.3961x; 1.0195x over previous
"""Trainium2 Bass kernel: 5x5 window median+variance denoise filter.

y = relu(x - noise_var/(var5x5(x)+1e-10) * (x - median5x5(x) + noise_bias))
with zero-padded 5x5 windows, unbiased variance (ddof=1).

Sharding: pure data parallel, B=16 images split 2-per-core across 8 cores.

v2: fp16 datapath. The median comparator network runs on DVE in fp16 to hit
the 2x_1p perf mode (all operands 2-byte, stride-1, 4B-aligned -- odd column
offsets are re-aligned via ACT-engine copies so no network op falls back to
1x). Squares, view-alignment copies, dtype conversions and the final relu
run on the otherwise-idle ACT engine. Variance accumulates in fp32 where it
matters (horizontal s25 sum, d/reciprocal path). Host pre-pads and converts
x to fp16; output returns fp16 and is upcast on host. Total numeric error
~1e-3 rel vs the 2e-2 harness gate.

Median via a pruned comparator network with shared column sorts:
  sort5 over the 5 dy-shifted planes (9 CE, shared by 5 horizontal windows)
  T = odd-even merge of adjacent sorted columns (13 CE, shared by 2 windows)
  final rank-12 selection from T(x-2), T(x), S(x+2) (35 CE, single-sided
  min/max pruned) -- 90 DVE min/max ops per full-image sweep, verified
  offline by exhaustive 0-1 principle.
"""
import numpy as np

import concourse.bass as bass  # noqa: F401
import concourse.mybir as mybir
from concourse import bacc, tile
from concourse.bass_utils import run_bass_kernel_spmd

F32 = mybir.dt.float32
F16 = mybir.dt.float16
ALU = mybir.AluOpType
ACTF = mybir.ActivationFunctionType

# (i, j, need_min, need_max) per structure; designed + 0/1-verified offline.
SORT5 = [(0, 1, 1, 1), (3, 4, 1, 1), (2, 4, 1, 1), (2, 3, 1, 1), (0, 3, 1, 1),
         (0, 2, 1, 1), (1, 4, 1, 1), (1, 3, 1, 1), (1, 2, 1, 1)]
T_CES = [(0, 5, 1, 1), (4, 9, 1, 1), (4, 5, 1, 1), (2, 7, 1, 1), (2, 4, 1, 1),
         (7, 5, 1, 1), (1, 6, 1, 1), (3, 8, 1, 1), (3, 6, 1, 1), (1, 2, 1, 1),
         (3, 4, 1, 1), (6, 7, 1, 1), (8, 5, 1, 1)]
F_CES = [(0, 10, 0, 1), (5, 15, 1, 0), (5, 10, 1, 1), (4, 14, 1, 1),
         (4, 5, 0, 1), (14, 10, 1, 0), (2, 12, 0, 1), (7, 17, 1, 0),
         (7, 12, 1, 1), (7, 5, 0, 1), (12, 14, 1, 1), (1, 11, 0, 1),
         (9, 19, 1, 0), (9, 11, 1, 1), (6, 16, 1, 1), (6, 9, 0, 1),
         (16, 11, 1, 0), (3, 13, 0, 1), (8, 18, 1, 0), (8, 13, 1, 1),
         (8, 9, 1, 1), (13, 16, 1, 0), (8, 5, 1, 1), (9, 12, 1, 1),
         (13, 14, 1, 1), (8, 20, 0, 1), (13, 24, 1, 0), (13, 20, 0, 1),
         (9, 22, 0, 1), (22, 20, 1, 0), (5, 21, 0, 1), (14, 21, 1, 0),
         (12, 23, 1, 0), (12, 14, 0, 1), (14, 22, 1, 0)]
F_OUT = 14


def schedule_ces(ces):
    """Reorder a CE list to avoid back-to-back producer->consumer ops on the
    in-order DVE (distance-1 RAW costs ~+150ns/op). Any permutation that
    preserves the relative order of CEs sharing a position has identical
    dataflow, so greedily pick ready CEs disjoint from the last emitted."""
    n = len(ces)
    preds = [set() for _ in range(n)]
    last_touch = {}
    for idx, ce in enumerate(ces):
        for p in ce[:2]:
            if p in last_touch:
                preds[idx].add(last_touch[p])
            last_touch[p] = idx
    emitted = [False] * n
    order = []
    hist = []          # positions of recently emitted CEs
    while len(order) < n:
        ready = [i for i in range(n) if not emitted[i]
                 and all(emitted[p] for p in preds[i])]
        pick = None
        for lookback in (2, 1, 0):
            recent = set().union(*hist[len(hist) - lookback:]) if lookback \
                else set()
            for i in ready:
                if not (set(ces[i][:2]) & recent):
                    pick = i
                    break
            if pick is not None:
                break
        emitted[pick] = True
        order.append(pick)
        hist.append(set(ces[pick][:2]))
    return [ces[i] for i in order]


SORT5 = schedule_ces(SORT5)
T_CES = schedule_ces(T_CES)
F_CES = schedule_ces(F_CES)

H = 512
W = 512
IMGS_PER_CORE = 2
N_CORES = 8
WIDE = W + 4          # 2-col halo each side
POOLW = WIDE + 2      # pool buffers padded so shifted views stay in-bounds
NBUF = 64             # cap on SBUF working buffers of [128, 2, POOLW] f16
NVB_COLS = 5          # nv, nb, c1=1/(24 nv), c2=1e-10/nv, sqrt(c1)/5


class BufPool:
    """Free-list over preallocated fixed SBUF tensors. Tile's dependency
    tracker makes reuse safe (WAR/RAW serialization on the same tensor)."""

    def __init__(self, nc):
        self.nc = nc
        self.bufs = []
        self.free = []

    def alloc(self):
        if self.free:
            return self.free.pop()
        idx = len(self.bufs)
        assert idx < NBUF, "SBUF buffer pool exhausted"
        t = self.nc.alloc_sbuf_tensor(f"wb{idx}", [128, 2, POOLW], F16).ap()
        self.bufs.append(t)
        return t

    def release(self, t):
        self.free.append(t)


class Wire:
    """SSA value living at column offset `off` of `buf`."""

    def __init__(self, buf, off, owned, pool, on_die=None):
        self.buf = buf
        self.off = off
        self.owned = owned      # release buf to pool when dead
        self.pool = pool
        self.reads_left = 0
        self.on_die = on_die

    def ap(self, width):
        return self.buf[:, :, self.off:self.off + width]

    def read_done(self):
        self.reads_left -= 1
        if self.reads_left == 0:
            self._die()

    def read_done_zero(self):
        if self.reads_left == 0:
            self._die()

    def _die(self):
        if self.owned:
            self.pool.release(self.buf)
        if self.on_die is not None:
            self.on_die()

    def detach_views(self, n_views):
        """Transfer buffer ownership to n_views future views; returns the
        on_die callback the views share. Call read_done() after to consume
        the terminal hold."""
        buf, owned, pool = self.buf, self.owned, self.pool
        self.owned = False
        state = {"n": n_views}

        def on_die():
            state["n"] -= 1
            if state["n"] == 0 and owned:
                pool.release(buf)
        return on_die


def run_stage(nc, pool, wires, ces, width, terminal_reads):
    """Emit one structure stage. A position's lifetime is split into segments
    at each rewrite; each Wire object gets the read count of its own segment
    so buffers release as soon as truly dead."""
    n = len(wires)
    # segs[i] = read counts per segment of position i (segment ends at a
    # write of i, which itself reads the old value).
    segs = [[] for _ in range(n)]
    cur = [0] * n
    for (a, b, nmin, nmax) in ces:
        cur[a] += 1
        cur[b] += 1
        if nmin:
            segs[a].append(cur[a])
            cur[a] = 0
        if nmax:
            segs[b].append(cur[b])
            cur[b] = 0
    for i in range(n):
        segs[i].append(cur[i] + terminal_reads.get(i, 0))

    seg_idx = [0] * n
    for i in range(n):
        wires[i].reads_left += segs[i][0]
        if segs[i][0] == 0:
            wires[i].read_done_zero()

    for (i, j, nmin, nmax) in ces:
        wi, wj = wires[i], wires[j]
        a = wi.ap(width)
        b = wj.ap(width)
        if nmin:
            lo = pool.alloc()
            nc.vector.tensor_tensor(lo[:, :, 0:width], a, b, ALU.min)
        if nmax:
            hi = pool.alloc()
            nc.vector.tensor_tensor(hi[:, :, 0:width], a, b, ALU.max)
        wi.read_done()
        wj.read_done()
        if nmin:
            seg_idx[i] += 1
            cnt = segs[i][seg_idx[i]]
            assert cnt > 0, "dead write (should be pruned offline)"
            wires[i] = Wire(lo, 0, True, pool)
            wires[i].reads_left = cnt
        if nmax:
            seg_idx[j] += 1
            cnt = segs[j][seg_idx[j]]
            assert cnt > 0, "dead write (should be pruned offline)"
            wires[j] = Wire(hi, 0, True, pool)
            wires[j].reads_left = cnt


def emit_chunk(nc, pool, f32bufs, tin, sq, out_tile, xa, ya, scal, img, half,
               probe=()):
    r0 = half * 256
    full = lambda t: t[:, :, :]
    nv_ap, nb_ap, c1_ap, c2_ap, sqc_ap = scal
    ident, P1, P2, V1sb, V2sb, q25c, s25c, tt = f32bufs

    # ---- loads: 5 dy-shifted fp16 tiles [128, 2, WIDE] from the pre-padded
    # shard (rows/cols already carry the 2-wide zero halo). Spread across the
    # three DGE queues (SP, ACT, GPSIMD) so transfers run in parallel. ----
    # NOTE: never issue DMA from the ACT queue -- its trigger would order
    # behind the whole per-chunk ACT program (which ends gated on DVE).
    if "noload" not in probe:
        for k, dy in enumerate(range(-2, 3)):
            for b in range(2):
                s = img * (H + 4) + r0 + b * 128 + dy + 2
                eng = nc.sync if (k + b) % 2 == 0 else nc.gpsimd
                eng.dma_start(tin[k][:, b, :], xa[s: s + 128, :])

    # ---- x-plane vertical sums on PE (reads tin only; runs immediately),
    # then ACT: V1 downconvert, squares, V2 path. ----
    if "novar" not in probe:
        for b in range(2):
            for lo, hi in ((0, 512), (512, WIDE)):
                for k in range(5):
                    nc.tensor.matmul(P1[b][:, lo:hi], ident[:, :],
                                     tin[k][:, b, lo:hi],
                                     start=(k == 0), stop=(k == 4))
            nc.scalar.copy(V1sb[:, b, :], P1[b][:, :])
        for k in range(5):
            nc.scalar.square(full(sq[k]), full(tin[k]))
        for b in range(2):
            for lo, hi in ((0, 512), (512, WIDE)):
                for k in range(5):
                    nc.tensor.matmul(P2[b][:, lo:hi], ident[:, :],
                                     sq[k][:, b, lo:hi],
                                     start=(k == 0), stop=(k == 4))
            nc.scalar.copy(V2sb[:, b, :], P2[b][:, :])
        # horizontal 5-sums back into the same (now free) PSUM banks
        for (Vsb, Pp) in ((V1sb, P1), (V2sb, P2)):
            for b in range(2):
                for dx in range(5):
                    nc.tensor.matmul(Pp[b][:, 0:W], ident[:, :],
                                     Vsb[:, b, dx:dx + W],
                                     start=(dx == 0), stop=(dx == 4))
    # ---- median network part 1: sort5 (all fp16 2x; DVE starts here) ----
    if "nomed" not in probe:
        s_wires = [Wire(tin[k], 0, False, pool) for k in range(5)]
        run_stage(nc, pool, s_wires, SORT5, WIDE, {k: 1 for k in range(5)})

        # Sorted column planes r_k feed three shifted views each: A_k = @0,
        # B_k = @1 (odd offset costs only ~+60ns/op -- cheaper than copies),
        # C_k = @4.
        t_wires = [None] * 10
        c_views = [None] * 5
        for k in range(5):
            rk = s_wires[k]
            od = rk.detach_views(3)
            t_wires[k] = Wire(rk.buf, rk.off + 0, False, pool, on_die=od)
            t_wires[k + 5] = Wire(rk.buf, rk.off + 1, False, pool, on_die=od)
            c_views[k] = Wire(rk.buf, rk.off + 4, False, pool, on_die=od)
            rk.read_done()      # consume terminal hold

    # q25c = c1*q25 + c2 ; s25c = (sqrt(c1)/5 * s25)^2 = c1*s25^2/25
    # (ACT, emitted after the B_k copies so those aren't delayed)
    if "novar" not in probe:
        for b in range(2):
            nc.scalar.activation(q25c[:, b, 0:W], P2[b][:, 0:W],
                                 ACTF.Identity, bias=c2_ap, scale=c1_ap)
            nc.scalar.activation(s25c[:, b, 0:W], P1[b][:, 0:W],
                                 ACTF.Square, scale=sqc_ap)

    # ---- median network part 2: T merge (width 516: tail lane is garbage
    # but never read downstream; keeps every op even-width/2x) ----
    if "nomed" not in probe:
        run_stage(nc, pool, t_wires, T_CES, WIDE, {j: 1 for j in range(10)})

    # ---- dd = q25c - s25c (GPSIMD; off the DVE stream) ; rcp = 1/dd ----
    if "novar" not in probe:
        nc.gpsimd.tensor_tensor(tt[:, :, 0:W], q25c[:, :, 0:W],
                                s25c[:, :, 0:W], ALU.subtract)
        nc.vector.reciprocal_approx_fast(out=q25c[:, :, 0:W],
                                         in_=tt[:, :, 0:W])
        rcp = pool.alloc()   # fp16 copy of reciprocal for the 2x formula ops
        nc.scalar.copy(rcp[:, :, 0:W], q25c[:, :, 0:W])

    # ---- median network part 3: final selection ----
    if "nomed" not in probe:
        f_wires = [None] * 25
        for j in range(10):
            tw = t_wires[j]
            od = tw.detach_views(2)
            f_wires[j] = Wire(tw.buf, tw.off + 0, False, pool, on_die=od)
            f_wires[j + 10] = Wire(tw.buf, tw.off + 2, False, pool, on_die=od)
            tw.read_done()
        for k in range(5):
            f_wires[20 + k] = c_views[k]

        run_stage(nc, pool, f_wires, F_CES, W, {F_OUT: 1})
        mid = f_wires[F_OUT]

    # ---- formula: y = relu(x - rcp*((x + nb) - mid)), all fp16 2x ----
    xc = tin[2][:, :, 2:2 + W]              # center plane = x
    u = pool.alloc()
    if "nomed" in probe:
        mid_ap = tin[0][:, :, 2:2 + W]
    else:
        mid_ap = mid.ap(W)
    nc.vector.scalar_tensor_tensor(u[:, :, 0:W], xc, nb_ap, mid_ap,
                                   ALU.add, ALU.subtract)
    if "nomed" not in probe:
        mid.read_done()
    if "novar" not in probe:
        nc.vector.tensor_tensor(u[:, :, 0:W], rcp[:, :, 0:W], u[:, :, 0:W],
                                ALU.mult)
        pool.release(rcp)
    nc.vector.tensor_tensor(u[:, :, 0:W], xc, u[:, :, 0:W], ALU.subtract)
    # per-block relu + store so block 0's writeback overlaps block 1's relu
    for b in range(2):
        nc.scalar.activation(out_tile[:, b, :], u[:, b, 0:W], ACTF.Relu)
        nc.gpsimd.dma_start(
            ya[img * H + r0 + b * 128: img * H + r0 + b * 128 + 128, :],
            out_tile[:, b, :],
        )
    pool.release(u)


def build_module(repeat=1, hw_loop=None, probe=()):
    nc = bacc.Bacc(
        "TRN2",
        target_bir_lowering=False,
        debug=False,
        enable_asserts=False,
        num_devices=N_CORES,
    )
    x = nc.dram_tensor("x", [IMGS_PER_CORE, H + 4, WIDE], F16,
                       kind="ExternalInput")
    nvb = nc.dram_tensor("nvb", [128, NVB_COLS], F32, kind="ExternalInput")
    idm = nc.dram_tensor("ident", [128, 128], F16, kind="ExternalInput")
    y = nc.dram_tensor("y", [IMGS_PER_CORE, H, W], F16, kind="ExternalOutput")

    xa = x.ap().flatten_outer_dims()    # [2*516, 516] fp16
    ya = y.ap().flatten_outer_dims()

    with tile.TileContext(nc) as tc:
        pool = BufPool(nc)
        nvb_t = nc.alloc_sbuf_tensor("nvb_t", [128, NVB_COLS], F32).ap()
        nc.sync.dma_start(nvb_t[:, :], nvb.ap()[:, :])
        scal = tuple(nvb_t[:, i:i + 1] for i in range(NVB_COLS))
        ident = nc.alloc_sbuf_tensor("ident_t", [128, 128], F16).ap()
        nc.sync.dma_start(ident[:, :], idm.ap()[:, :])

        # double-buffered input/square/output tiles (chunk parity)
        tin = [[nc.alloc_sbuf_tensor(f"tin{p}_{k}", [128, 2, WIDE], F16).ap()
                for k in range(5)] for p in range(2)]
        sq = [[nc.alloc_sbuf_tensor(f"sq{p}_{k}", [128, 2, WIDE], F16).ap()
               for k in range(5)] for p in range(2)]
        out_t = [nc.alloc_sbuf_tensor(f"out{p}", [128, 2, W], F16).ap()
                 for p in range(2)]
        P1 = [nc.alloc_psum_tensor(f"P1b{b}", [128, WIDE], F32).ap()
              for b in range(2)]
        P2 = [nc.alloc_psum_tensor(f"P2b{b}", [128, WIDE], F32).ap()
              for b in range(2)]
        V1sb = nc.alloc_sbuf_tensor("V1sb", [128, 2, WIDE], F16).ap()
        V2sb = nc.alloc_sbuf_tensor("V2sb", [128, 2, WIDE], F16).ap()
        q25c = nc.alloc_sbuf_tensor("q25c", [128, 2, W], F32).ap()
        s25c = nc.alloc_sbuf_tensor("s25c", [128, 2, W], F32).ap()
        tt = nc.alloc_sbuf_tensor("ttb", [128, 2, W], F32).ap()
        f32bufs = (ident, P1, P2, V1sb, V2sb, q25c, s25c, tt)

        def body():
            for _ in range(repeat):
                for ci in range(2 * IMGS_PER_CORE):
                    img, half = divmod(ci, 2)
                    p = ci & 1
                    emit_chunk(nc, pool, f32bufs, tin[p], sq[p], out_t[p],
                               xa, ya, scal, img, half, probe=probe)

        if hw_loop is None:
            body()
        else:
            with tc.For_i(0, hw_loop, 1):
                body()

    nc.compile()
    return nc


_MODULE = None


def _get_module():
    global _MODULE
    if _MODULE is None:
        _MODULE = build_module()
    return _MODULE


def make_in_maps(x, nv, nb):
    """Host-side prep: pad + fp16-convert x, build per-core input maps."""
    nvb = np.empty((128, NVB_COLS), np.float32)
    c1 = 1.0 / (24.0 * nv)
    nvb[:, 0] = nv
    nvb[:, 1] = nb
    nvb[:, 2] = c1
    nvb[:, 3] = 1e-10 / nv
    nvb[:, 4] = np.sqrt(c1) / 5.0
    ident = np.eye(128, dtype=np.float16)

    B = x.shape[0]
    xpad = np.zeros((B, H + 4, WIDE), np.float16)
    xpad[:, 2:2 + H, 2:2 + W] = x[:, 0]
    in_maps = []
    for c in range(N_CORES):
        shard = np.ascontiguousarray(
            xpad[c * IMGS_PER_CORE:(c + 1) * IMGS_PER_CORE])
        in_maps.append({"x": shard, "nvb": nvb, "ident": ident})
    return in_maps


def kernel(x, noise_var, noise_bias):
    x = np.ascontiguousarray(np.asarray(x, dtype=np.float32))
    nv = float(np.asarray(noise_var).reshape(-1)[0])
    nb = float(np.asarray(noise_bias).reshape(-1)[0])
    B = x.shape[0]
    assert x.shape == (B, 1, H, W) and B == N_CORES * IMGS_PER_CORE

    nc = _get_module()
    in_maps = make_in_maps(x, nv, nb)
    res = run_bass_kernel_spmd(nc, in_maps, core_ids=list(range(N_CORES)))
    y = np.empty((B, 1, H, W), np.float32)
    for c in range(N_CORES):
        y[c * IMGS_PER_CORE:(c + 1) * IMGS_PER_CORE, 0] = \
            res.results[c]["y"].astype(np.float32)
    return y
